# revision 16
# baseline (speedup 1.0000x reference)
"""Trainium2 Bass kernel for nn_AstraloraLayer: y = x @ A.T (the low-rank
surrogate path cancels in the forward value).

Sharding: data-parallel over tokens. Each of the 8 cores computes
y[c] = x[c] @ A.T for its [2048, 4096] token shard; A = w.reshape(4096, 4096)
is replicated. No collectives.

Per-core kernel: Y.T[o, t] = sum_k A.T[k, o] * X.T[k, t]. Hybrid precision
over the contraction: k-tiles 0..21 run as fp16 TensorE matmuls (1 cycle/row),
k-tiles 22..31 run as fp8e4 DoubleRow matmuls (2 fp8 weights per PE cell,
0.5 cycles/row). The fp8 operands are pre-scaled (x*8, A*512); the fp16 A is
pre-scaled by 4096 so every matmul accumulates 4096*y in PSUM, and the
PSUM->SBUF drain copy multiplies by 2^-12. Measured rel err ~1.8e-2 vs the
2e-2 gate.

Three token phases (512, 512, 1024) so the first output tiles only wait on a
quarter of X.T; within each output tile the fp8 DoubleRow matmuls (whose
operands land first) run before the fp16 stream. fp16 A.T streams once per
phase, fp8 A.T (4MB) is resident. Warm-up matmuls on scratch SBUF trip the
HAM clock gate before real data lands; the last output tile drains in four
256-token groups across two HWDGE queues to shorten the end-of-kernel tail.
"""

import sys

import numpy as np

if "/opt/trn_rl_repo" not in sys.path:
    sys.path.insert(0, "/opt/trn_rl_repo")

D = 4096          # d_inp == d_out
TOK = 2048        # tokens per core (8 * 2048 total)
N_CORES = 8
P = 128           # partitions
KH = D // P       # 32 k-tiles over the contraction dim
KH16 = 22         # k-tiles 0..21 in fp16
NDR = (KH - KH16) // 2  # 5 DoubleRow pairs for k-tiles 22..31
NOT = D // P      # 32 output tiles
PHASES = [(0, 512), (512, 512), (1024, 1024)]
WARMUP_MMS = 20   # scratch matmuls to warm the PE clock before data lands

SX = 8.0          # fp8 x scale
SA = 512.0        # fp8 A scale
SH = SX * SA      # fp16 A pre-scale; PSUM holds SH * y
INV = 1.0 / SH

_COMPILED = None


def _build():
    import concourse.mybir as mybir
    import concourse.tile as tile
    from concourse import bacc

    f16 = mybir.dt.float16
    f8 = mybir.dt.float8e4
    f32 = mybir.dt.float32
    DR = mybir.MatmulPerfMode.DoubleRow

    nc = bacc.Bacc("TRN2", target_bir_lowering=False)

    # xth[p, kh, t] = x[t, kh*128 + p]                          (kh < KH16)
    xth_ext = nc.declare_dram_parameter("xth", [P, KH16, TOK], f16, isOutput=False)
    # xt8[p, j, s, t] = x[t, (KH16+2j+s)*128 + p] * SX
    xt8_ext = nc.declare_dram_parameter("xt8", [P, NDR, 2, TOK], f8, isOutput=False)
    # ath[p, ot, kh, o] = A[ot*128 + o, kh*128 + p] * SH        (kh < KH16)
    ath_ext = nc.declare_dram_parameter("ath", [P, NOT, KH16, P], f16, isOutput=False)
    # at8[p, ot, j, s, o] = A[ot*128 + o, (KH16+2j+s)*128 + p] * SA
    at8_ext = nc.declare_dram_parameter(
        "at8", [P, NOT, NDR, 2, P], f8, isOutput=False
    )
    # out: Y.T [o, t]
    out_ext = nc.declare_dram_parameter("out", [D, TOK], f32, isOutput=True)

    # fp16 X.T chunk plan per phase (kh tiles per chunk)
    CHUNK_PLAN = [[1, 3, 4, 4, 4, 4, 2], [8, 8, 6], [11, 11]]

    with tile.TileContext(nc) as tc:
        with (
            tc.tile_pool(name="wu", bufs=1) as wu_pool,
            tc.tile_pool(name="a8", bufs=1) as a8_pool,
            tc.tile_pool(name="xt", bufs=1) as xt_pool,
            tc.tile_pool(name="at", bufs=4) as at_pool,
            tc.tile_pool(name="ps5", bufs=2, space="PSUM") as ps5_pool,
            tc.tile_pool(name="ps10", bufs=2, space="PSUM") as ps10_pool,
            tc.tile_pool(name="pst", bufs=2, space="PSUM") as pst_pool,
            tc.tile_pool(name="ys", bufs=3) as ys_pool,
        ):
            # Warm-up matmuls on scratch SBUF: keep the PE busy while the
            # first DMAs land, and trip the HAM clock gate to 2.4 GHz
            # before the first real matmul issues.
            wu = wu_pool.tile([P, 256], f16, tag="wu", name="wu")
            nc.vector.memset(wu[:], 0.25)
            wps = ps10_pool.tile([P, 1024], f32, tag="ps10", name="ps10")
            for _ in range(WARMUP_MMS):
                nc.tensor.matmul(
                    wps[:, 0:P], wu[:, 0:P], wu[:, P : 2 * P], start=True, stop=True
                )

            # fp8 A.T is A-only and small (5MB): resident, loaded in 4
            # sub-tiles of 8 ot on the scalar HWDGE queue so the first DR
            # matmuls only wait on 1.25MB. Only sub 0 loads up front; subs
            # 1-3 are deferred into the phase-0 ot loop so they don't steal
            # DMA bandwidth from the fp16 A.T stream during startup.
            a8_subs = []
            for g in range(4):
                s = a8_pool.tile(
                    [P, 8, NDR, 2, P], f8, tag=f"a8g{g}", name=f"a8g{g}"
                )
                if g == 0:
                    nc.scalar.dma_start(
                        out=s[:], in_=at8_ext[:, 0:8, :, :, :]
                    )
                a8_subs.append(s)

            # X.T loads ride the gpsimd DMA queue, separate from the A.T
            # stream on the sync queue. Each chunk is its own tile so matmuls
            # only wait on the chunk they actually read. Only phase 0's DMAs
            # issue up front (first fp16 chunk ahead of the fp8 slice so the
            # very first matmul starts early); later phases' loads are
            # deferred into the ot loop so the startup window's DMA
            # bandwidth belongs to the fp16 A.T stream.
            xth_sb, xth_map, x8_sb = [], [], []
            for ph, (pt0, ptn) in enumerate(PHASES):
                x8 = xt_pool.tile(
                    [P, NDR, 2, ptn], f8, tag=f"x8p{ph}", name=f"x8p{ph}"
                )
                x8_sb.append(x8)
                chunks, kmap, kh0 = [], [], 0
                for c, ch in enumerate(CHUNK_PLAN[ph]):
                    t = xt_pool.tile(
                        [P, ch, ptn], f16, tag=f"xtp{ph}c{c}", name=f"xtp{ph}c{c}"
                    )
                    for r in range(ch):
                        kmap.append((c, r))
                    chunks.append(t)
                    kh0 += ch
                xth_sb.append(chunks)
                xth_map.append(kmap)

            def load_xt(ph, upto_chunk=None):
                pt0, ptn = PHASES[ph]
                kh0 = 0
                for c, ch in enumerate(CHUNK_PLAN[ph]):
                    if upto_chunk is None or c <= upto_chunk:
                        nc.gpsimd.dma_start(
                            out=xth_sb[ph][c][:],
                            in_=xth_ext[:, kh0 : kh0 + ch, pt0 : pt0 + ptn],
                        )
                    kh0 += ch
                if upto_chunk is None:
                    nc.gpsimd.dma_start(
                        out=x8_sb[ph][:], in_=xt8_ext[:, :, :, pt0 : pt0 + ptn]
                    )

            # phase 0: chunks 0-1, then the fp8 slice, then the rest
            pt0, ptn = PHASES[0]
            load_xt(0, upto_chunk=1)
            nc.gpsimd.dma_start(
                out=x8_sb[0][:], in_=xt8_ext[:, :, :, pt0 : pt0 + ptn]
            )
            kh0 = CHUNK_PLAN[0][0] + CHUNK_PLAN[0][1]
            for c in range(2, len(CHUNK_PLAN[0])):
                ch = CHUNK_PLAN[0][c]
                nc.gpsimd.dma_start(
                    out=xth_sb[0][c][:],
                    in_=xth_ext[:, kh0 : kh0 + ch, pt0 : pt0 + ptn],
                )
                kh0 += ch

            for ph, (pt0, ptn) in enumerate(PHASES):
                for ot in range(NOT):
                    # Deferred loads of the remaining resident fp8 A.T subs,
                    # two output-tiles ahead of first use.
                    if ph == 0 and ot in (6, 14, 22):
                        g = ot // 8 + 1
                        nc.scalar.dma_start(
                            out=a8_subs[g][:],
                            in_=at8_ext[:, g * 8 : (g + 1) * 8, :, :, :],
                        )
                    # Deferred X.T loads for the later phases, well ahead of
                    # first use but clear of the startup bandwidth crunch.
                    if ph == 0 and ot == 8:
                        load_xt(1)
                    if ph == 0 and ot == 18:
                        load_xt(2)
                    # First fp16 A.T tile of the run arrives kh-sliced so the
                    # first fp16 matmuls only wait on a 64KB load.
                    if ph == 0 and ot == 0:
                        at_subs, kh0 = [], 0
                        for si, ch in enumerate((2, 6, 7, 7)):
                            s = at_pool.tile(
                                [P, ch, P], f16, tag=f"at0s{si}", name="at_s"
                            )
                            nc.sync.dma_start(
                                out=s[:], in_=ath_ext[:, ot, kh0 : kh0 + ch, :]
                            )
                            for r in range(ch):
                                at_subs.append((s, r))
                            kh0 += ch
                    else:
                        at_t = at_pool.tile([P, KH16, P], f16, tag="at", name="at_t")
                        nc.sync.dma_start(out=at_t[:], in_=ath_ext[:, ot, :, :])
                        at_subs = [(at_t, kh) for kh in range(KH16)]
                    a8t = a8_subs[ot // 8]
                    o8 = ot % 8

                    # The very last output tile runs as 4 independent
                    # 256-token accumulation groups so its drain (copy +
                    # store + DMA receipt) pipelines against its own matmuls
                    # instead of all landing after the final matmul.
                    last = ph == len(PHASES) - 1 and ot == NOT - 1
                    if last:
                        for g in range(4):
                            t0, t1 = g * 256, (g + 1) * 256
                            pst = pst_pool.tile([P, 256], f32, tag="pst", name="pst")
                            for kh in range(KH16):
                                c, r = xth_map[ph][kh]
                                a_t, a_r = at_subs[kh]
                                nc.tensor.matmul(
                                    pst[:],
                                    a_t[:, a_r, :],
                                    xth_sb[ph][c][:, r, t0:t1],
                                    start=(kh == 0),
                                    stop=False,
                                )
                            for j in range(NDR):
                                nc.tensor.matmul(
                                    pst[:],
                                    a8t[:, o8, j, :, :],
                                    x8_sb[ph][:, j, :, t0:t1],
                                    start=False,
                                    stop=(j == NDR - 1),
                                    perf_mode=DR,
                                )
                            ys = ys_pool.tile([P, 256], f32, tag="ys", name="ys256")
                            nc.vector.tensor_scalar_mul(ys[:], pst[:], INV)
                            eng = nc.sync if g % 2 == 1 else nc.scalar
                            eng.dma_start(
                                out=out_ext[
                                    ot * P : (ot + 1) * P, pt0 + t0 : pt0 + t1
                                ],
                                in_=ys[:],
                            )
                        continue

                    pool = ps10_pool if ptn == 1024 else ps5_pool
                    ps = pool.tile(
                        [P, ptn], f32, tag=f"ps{10 if ptn == 1024 else 5}",
                        name="ps",
                    )
                    # Phase 0 runs the fp8 DoubleRow matmuls first (their
                    # operands land before the fp16 X.T stream); later phases
                    # run fp16 first so each accumulation group opens with
                    # the cheap 128-col LDWEIGHTS (the group-start weight
                    # load doesn't hide behind the previous matmul).
                    def dr_mms(first):
                        for j in range(NDR):
                            for h in range(ptn // 512):
                                nc.tensor.matmul(
                                    ps[:, h * 512 : (h + 1) * 512],
                                    a8t[:, o8, j, :, :],
                                    x8_sb[ph][:, j, :, h * 512 : (h + 1) * 512],
                                    start=(first and j == 0),
                                    stop=(not first and j == NDR - 1),
                                    perf_mode=DR,
                                )

                    def f16_mms(first):
                        for kh in range(KH16):
                            c, r = xth_map[ph][kh]
                            a_t, a_r = at_subs[kh]
                            for h in range(ptn // 512):
                                nc.tensor.matmul(
                                    ps[:, h * 512 : (h + 1) * 512],
                                    a_t[:, a_r, :],
                                    xth_sb[ph][c][:, r, h * 512 : (h + 1) * 512],
                                    start=(first and kh == 0),
                                    stop=(not first and kh == KH16 - 1),
                                )

                    f16_mms(True)
                    dr_mms(False)
                    # Output drain: the PSUM->SBUF copy applies the 2^-12
                    # descale; stores ride the scalar HWDGE queue (A.T loads
                    # own the sync queue).
                    ys = ys_pool.tile([P, ptn], f32, tag="ys", name=f"ys{ptn}")
                    nc.vector.tensor_scalar_mul(ys[:], ps[:], INV)
                    nc.scalar.dma_start(
                        out=out_ext[ot * P : (ot + 1) * P, pt0 : pt0 + ptn],
                        in_=ys[:],
                    )

    nc.compile()
    return nc


def _get_compiled():
    global _COMPILED
    if _COMPILED is None:
        _COMPILED = _build()
    return _COMPILED


def _f8np():
    import ml_dtypes

    return ml_dtypes.float8_e4m3


def _pack_a(w):
    A = np.asarray(w, dtype=np.float32).reshape(D, D)
    # fp16 part: [p, ot, kh, o] = A[ot*128+o, kh*128+p] * SH for kh < KH16
    Ah = (A[:, : KH16 * P] * SH).reshape(NOT, P, KH16, P)
    ath = np.ascontiguousarray(Ah.transpose(3, 0, 2, 1), dtype=np.float16)
    # fp8 part: [p, ot, j, s, o] = A[ot*128+o, (KH16+2j+s)*128+p] * SA
    A8 = (A[:, KH16 * P :] * SA).reshape(NOT, P, NDR, 2, P)
    at8 = np.ascontiguousarray(A8.transpose(4, 0, 2, 3, 1)).astype(_f8np())
    return ath, at8


def _pack_x(xc):
    xc = np.asarray(xc, dtype=np.float32)
    # fp16 part: [p, kh, t] = x[t, kh*128+p]
    Xh = xc[:, : KH16 * P].reshape(TOK, KH16, P)
    xth = np.ascontiguousarray(Xh.transpose(2, 1, 0), dtype=np.float16)
    # fp8 part: [p, j, s, t]
    X8 = (xc[:, KH16 * P :] * SX).reshape(TOK, NDR, 2, P)
    xt8 = np.ascontiguousarray(X8.transpose(3, 1, 2, 0)).astype(_f8np())
    return xth, xt8


def _prep_in_maps(inputs):
    x = np.asarray(inputs["x"])
    ath, at8 = _pack_a(np.asarray(inputs["w"]))
    in_maps = []
    for c in range(N_CORES):
        xth, xt8 = _pack_x(x[c])
        in_maps.append({"xth": xth, "xt8": xt8, "ath": ath, "at8": at8})
    return in_maps


def kernel(x, w, U, S, V):
    from concourse.bass_utils import run_bass_kernel_spmd

    assert x.shape == (N_CORES, TOK, D)
    nc = _get_compiled()
    in_maps = _prep_in_maps({"x": x, "w": w})

    res = run_bass_kernel_spmd(nc, in_maps, core_ids=list(range(N_CORES)))

    y = np.empty((N_CORES, TOK, D), dtype=np.float32)
    for c in range(N_CORES):
        y[c] = res.results[c]["out"].T
    return y


# revision 18
# speedup vs baseline: 1.1869x; 1.1869x over previous
"""Trainium2 Bass kernel for nn_AstraloraLayer: y = x @ A.T (the low-rank
surrogate path cancels in the forward value).

Sharding: data-parallel over tokens. Each of the 8 cores computes
y[c] = x[c] @ A.T for its [2048, 4096] token shard; A = w.reshape(4096, 4096)
is replicated. No collectives.

Per-core kernel: Y.T[o, t] = sum_k A.T[k, o] * X.T[k, t]. Hybrid precision
over the contraction: k-tiles 0..21 run as fp16 TensorE matmuls (1 cycle/row),
k-tiles 22..31 run as fp8e4 DoubleRow matmuls (2 fp8 weights per PE cell,
0.5 cycles/row). The fp8 operands are pre-scaled (x*8, A*512); the fp16 A is
pre-scaled by 4096 so every matmul accumulates 4096*y in PSUM, and the
PSUM->SBUF drain copy multiplies by 2^-12. Measured rel err ~1.8e-2 vs the
2e-2 gate.

Three token phases (512, 512, 1024) so the first output tiles only wait on a
quarter of X.T; within each output tile the fp8 DoubleRow matmuls (whose
operands land first) run before the fp16 stream. fp16 A.T streams once per
phase, fp8 A.T (4MB) is resident. Warm-up matmuls on scratch SBUF trip the
HAM clock gate before real data lands; the last output tile drains in four
256-token groups across two HWDGE queues to shorten the end-of-kernel tail.
"""

import sys

import numpy as np

if "/opt/trn_rl_repo" not in sys.path:
    sys.path.insert(0, "/opt/trn_rl_repo")

D = 4096          # d_inp == d_out
TOK = 2048        # tokens per core (8 * 2048 total)
N_CORES = 8
P = 128           # partitions
KH = D // P       # 32 k-tiles over the contraction dim
KH16 = 22         # k-tiles 0..21 in fp16
NDR = (KH - KH16) // 2  # 5 DoubleRow pairs for k-tiles 22..31
NOT = D // P      # 32 output tiles
PHASES = [(0, 512), (512, 512), (1024, 1024)]
WARMUP_MMS = 48   # scratch matmuls to warm the PE clock before data lands

SX = 8.0          # fp8 x scale
SA = 512.0        # fp8 A scale
SH = SX * SA      # fp16 A pre-scale; PSUM holds SH * y
INV = 1.0 / SH

_COMPILED = None


def _build():
    import concourse.mybir as mybir
    import concourse.tile as tile
    from concourse import bacc

    f16 = mybir.dt.float16
    f8 = mybir.dt.float8e4
    f32 = mybir.dt.float32
    DR = mybir.MatmulPerfMode.DoubleRow

    nc = bacc.Bacc("TRN2", target_bir_lowering=False)

    # xth[p, kh, t] = x[t, kh*128 + p]                          (kh < KH16)
    xth_ext = nc.declare_dram_parameter("xth", [P, KH16, TOK], f16, isOutput=False)
    # xt8[p, j, s, t] = x[t, (KH16+2j+s)*128 + p] * SX
    xt8_ext = nc.declare_dram_parameter("xt8", [P, NDR, 2, TOK], f8, isOutput=False)
    # ath[p, ot, kh, o] = A[ot*128 + o, kh*128 + p] * SH        (kh < KH16)
    ath_ext = nc.declare_dram_parameter("ath", [P, NOT, KH16, P], f16, isOutput=False)
    # at8[p, ot, j, s, o] = A[ot*128 + o, (KH16+2j+s)*128 + p] * SA
    at8_ext = nc.declare_dram_parameter(
        "at8", [P, NOT, NDR, 2, P], f8, isOutput=False
    )
    # out: Y.T [o, t]
    out_ext = nc.declare_dram_parameter("out", [D, TOK], f32, isOutput=True)

    # fp16 X.T chunk plan per phase (kh tiles per chunk)
    CHUNK_PLAN = [[1, 3, 4, 4, 4, 4, 2], [8, 8, 6], [11, 11]]

    with tile.TileContext(nc) as tc:
        with (
            tc.tile_pool(name="wu", bufs=1) as wu_pool,
            tc.tile_pool(name="a8", bufs=1) as a8_pool,
            tc.tile_pool(name="xt", bufs=1) as xt_pool,
            tc.tile_pool(name="at", bufs=4) as at_pool,
            tc.tile_pool(name="ps5", bufs=2, space="PSUM") as ps5_pool,
            tc.tile_pool(name="ps10", bufs=2, space="PSUM") as ps10_pool,
            tc.tile_pool(name="pst", bufs=2, space="PSUM") as pst_pool,
            tc.tile_pool(name="ys", bufs=3) as ys_pool,
        ):
            # Warm-up matmuls on scratch SBUF: keep the PE busy while the
            # first DMAs land, and trip the HAM clock gate to 2.4 GHz
            # before the first real matmul issues.
            wu = wu_pool.tile([P, 256], f16, tag="wu", name="wu")
            nc.vector.memset(wu[:], 0.25)
            wps = ps10_pool.tile([P, 1024], f32, tag="ps10", name="ps10")
            for _ in range(WARMUP_MMS):
                nc.tensor.matmul(
                    wps[:, 0:P], wu[:, 0:P], wu[:, P : 2 * P], start=True, stop=True
                )

            # fp8 A.T is A-only and small (5MB): resident, loaded in 4
            # sub-tiles of 8 ot on the scalar HWDGE queue so the first DR
            # matmuls only wait on 1.25MB. Only sub 0 loads up front; subs
            # 1-3 are deferred into the phase-0 ot loop so they don't steal
            # DMA bandwidth from the fp16 A.T stream during startup.
            a8_0a = a8_pool.tile([P, 2, NDR, 2, P], f8, tag="a8g0a", name="a8g0a")
            nc.scalar.dma_start(out=a8_0a[:], in_=at8_ext[:, 0:2, :, :, :])
            a8_0b = a8_pool.tile([P, 6, NDR, 2, P], f8, tag="a8g0b", name="a8g0b")
            a8_subs = [None]
            for g in range(1, 4):
                s = a8_pool.tile(
                    [P, 8, NDR, 2, P], f8, tag=f"a8g{g}", name=f"a8g{g}"
                )
                a8_subs.append(s)

            # X.T loads ride the gpsimd DMA queue, separate from the A.T
            # stream on the sync queue. Per phase: the fp8 slice first (the
            # DR matmuls run first within each ot), then the fp16 chunks.
            # Each chunk is its own tile so matmuls only wait on the chunk
            # they actually read.
            xth_sb, xth_map, x8_sb = [], [], []
            for ph, (pt0, ptn) in enumerate(PHASES):
                x8 = xt_pool.tile(
                    [P, NDR, 2, ptn], f8, tag=f"x8p{ph}", name=f"x8p{ph}"
                )
                x8_sb.append(x8)
                chunks, kmap, kh0 = [], [], 0
                for c, ch in enumerate(CHUNK_PLAN[ph]):
                    t = xt_pool.tile(
                        [P, ch, ptn], f16, tag=f"xtp{ph}c{c}", name=f"xtp{ph}c{c}"
                    )
                    for r in range(ch):
                        kmap.append((c, r))
                    chunks.append(t)
                    kh0 += ch
                xth_sb.append(chunks)
                xth_map.append(kmap)

            def issue_xt(ph, chunk_ids, with_x8_first=False, with_x8_last=False):
                pt0, ptn = PHASES[ph]
                if with_x8_first:
                    nc.gpsimd.dma_start(
                        out=x8_sb[ph][:], in_=xt8_ext[:, :, :, pt0 : pt0 + ptn]
                    )
                bounds = []
                kh0 = 0
                for ch in CHUNK_PLAN[ph]:
                    bounds.append((kh0, ch))
                    kh0 += ch
                for c in chunk_ids:
                    k0, ch = bounds[c]
                    nc.gpsimd.dma_start(
                        out=xth_sb[ph][c][:],
                        in_=xth_ext[:, k0 : k0 + ch, pt0 : pt0 + ptn],
                    )
                if with_x8_last:
                    nc.gpsimd.dma_start(
                        out=x8_sb[ph][:], in_=xt8_ext[:, :, :, pt0 : pt0 + ptn]
                    )

            # phase 0: fp16 chunks 0-1 first (the first matmuls are fp16),
            # then the fp8 slice, then the rest; later phases eager behind.
            issue_xt(0, [0, 1], with_x8_last=True)
            issue_xt(0, list(range(2, len(CHUNK_PLAN[0]))))
            issue_xt(1, list(range(len(CHUNK_PLAN[1]))), with_x8_first=True)
            issue_xt(2, list(range(len(CHUNK_PLAN[2]))), with_x8_first=True)

            for ph, (pt0, ptn) in enumerate(PHASES):
                for ot in range(NOT):
                    # Deferred loads of the remaining resident fp8 A.T subs,
                    # two output-tiles ahead of first use.
                    if ph == 0 and ot == 1:
                        nc.scalar.dma_start(
                            out=a8_0b[:], in_=at8_ext[:, 2:8, :, :, :]
                        )
                    if ph == 0 and ot in (6, 14, 22):
                        g = ot // 8 + 1
                        nc.scalar.dma_start(
                            out=a8_subs[g][:],
                            in_=at8_ext[:, g * 8 : (g + 1) * 8, :, :, :],
                        )
                    # First fp16 A.T tile of the run arrives kh-sliced so the
                    # first fp16 matmuls only wait on a 64KB load.
                    if ph == 0 and ot == 0:
                        at_subs, kh0 = [], 0
                        for si, ch in enumerate((2, 6, 7, 7)):
                            s = at_pool.tile(
                                [P, ch, P], f16, tag=f"at0s{si}", name="at_s"
                            )
                            nc.sync.dma_start(
                                out=s[:], in_=ath_ext[:, ot, kh0 : kh0 + ch, :]
                            )
                            for r in range(ch):
                                at_subs.append((s, r))
                            kh0 += ch
                    else:
                        at_t = at_pool.tile([P, KH16, P], f16, tag="at", name="at_t")
                        nc.sync.dma_start(out=at_t[:], in_=ath_ext[:, ot, :, :])
                        at_subs = [(at_t, kh) for kh in range(KH16)]
                    if ot < 2:
                        a8t, o8 = a8_0a, ot
                    elif ot < 8:
                        a8t, o8 = a8_0b, ot - 2
                    else:
                        a8t, o8 = a8_subs[ot // 8], ot % 8

                    # The very last output tile runs as 4 independent
                    # 256-token accumulation groups so its drain (copy +
                    # store + DMA receipt) pipelines against its own matmuls
                    # instead of all landing after the final matmul.
                    last = ph == len(PHASES) - 1 and ot == NOT - 1
                    if last:
                        GRP = [(0, 320), (320, 320), (640, 320), (960, 64)]
                        for g in range(4):
                            t0, t1 = GRP[g][0], GRP[g][0] + GRP[g][1]
                            pst = pst_pool.tile(
                                [P, t1 - t0], f32, tag="pst", name="pst"
                            )
                            for j in range(NDR):
                                nc.tensor.matmul(
                                    pst[:],
                                    a8t[:, o8, j, :, :],
                                    x8_sb[ph][:, j, :, t0:t1],
                                    start=(j == 0),
                                    stop=False,
                                    perf_mode=DR,
                                )
                            for kh in range(KH16):
                                c, r = xth_map[ph][kh]
                                a_t, a_r = at_subs[kh]
                                nc.tensor.matmul(
                                    pst[:],
                                    a_t[:, a_r, :],
                                    xth_sb[ph][c][:, r, t0:t1],
                                    start=False,
                                    stop=(kh == KH16 - 1),
                                )
                            ys = ys_pool.tile(
                                [P, t1 - t0], f32, tag="ys", name="ysg"
                            )
                            nc.vector.tensor_scalar_mul(ys[:], pst[:], INV)
                            eng = nc.sync if g % 2 == 1 else nc.scalar
                            eng.dma_start(
                                out=out_ext[
                                    ot * P : (ot + 1) * P, pt0 + t0 : pt0 + t1
                                ],
                                in_=ys[:],
                            )
                        continue

                    pool = ps10_pool if ptn == 1024 else ps5_pool
                    ps = pool.tile(
                        [P, ptn], f32, tag=f"ps{10 if ptn == 1024 else 5}",
                        name="ps",
                    )
                    # Phase 0 runs the fp8 DoubleRow matmuls first (their
                    # operands land before the fp16 X.T stream); later phases
                    # run fp16 first so each accumulation group opens with
                    # the cheap 128-col LDWEIGHTS (the group-start weight
                    # load doesn't hide behind the previous matmul).
                    def dr_mms(first):
                        for j in range(NDR):
                            for h in range(ptn // 512):
                                nc.tensor.matmul(
                                    ps[:, h * 512 : (h + 1) * 512],
                                    a8t[:, o8, j, :, :],
                                    x8_sb[ph][:, j, :, h * 512 : (h + 1) * 512],
                                    start=(first and j == 0),
                                    stop=(not first and j == NDR - 1),
                                    perf_mode=DR,
                                )

                    def f16_mms(first):
                        for kh in range(KH16):
                            c, r = xth_map[ph][kh]
                            a_t, a_r = at_subs[kh]
                            for h in range(ptn // 512):
                                nc.tensor.matmul(
                                    ps[:, h * 512 : (h + 1) * 512],
                                    a_t[:, a_r, :],
                                    xth_sb[ph][c][:, r, h * 512 : (h + 1) * 512],
                                    start=(first and kh == 0),
                                    stop=(not first and kh == KH16 - 1),
                                )

                    f16_mms(True)
                    dr_mms(False)
                    # Output drain: the PSUM->SBUF copy applies the 2^-12
                    # descale; stores ride the scalar HWDGE queue (A.T loads
                    # own the sync queue).
                    ys = ys_pool.tile([P, ptn], f32, tag="ys", name=f"ys{ptn}")
                    nc.vector.tensor_scalar_mul(ys[:], ps[:], INV)
                    nc.scalar.dma_start(
                        out=out_ext[ot * P : (ot + 1) * P, pt0 : pt0 + ptn],
                        in_=ys[:],
                    )

    nc.compile()
    return nc


def _get_compiled():
    global _COMPILED
    if _COMPILED is None:
        _COMPILED = _build()
    return _COMPILED


def _f8np():
    import ml_dtypes

    return ml_dtypes.float8_e4m3


def _pack_a(w):
    A = np.asarray(w, dtype=np.float32).reshape(D, D)
    # fp16 part: [p, ot, kh, o] = A[ot*128+o, kh*128+p] * SH for kh < KH16
    Ah = (A[:, : KH16 * P] * SH).reshape(NOT, P, KH16, P)
    ath = np.ascontiguousarray(Ah.transpose(3, 0, 2, 1), dtype=np.float16)
    # fp8 part: [p, ot, j, s, o] = A[ot*128+o, (KH16+2j+s)*128+p] * SA
    A8 = (A[:, KH16 * P :] * SA).reshape(NOT, P, NDR, 2, P)
    at8 = np.ascontiguousarray(A8.transpose(4, 0, 2, 3, 1)).astype(_f8np())
    return ath, at8


def _pack_x(xc):
    xc = np.asarray(xc, dtype=np.float32)
    # fp16 part: [p, kh, t] = x[t, kh*128+p]
    Xh = xc[:, : KH16 * P].reshape(TOK, KH16, P)
    xth = np.ascontiguousarray(Xh.transpose(2, 1, 0), dtype=np.float16)
    # fp8 part: [p, j, s, t]
    X8 = (xc[:, KH16 * P :] * SX).reshape(TOK, NDR, 2, P)
    xt8 = np.ascontiguousarray(X8.transpose(3, 1, 2, 0)).astype(_f8np())
    return xth, xt8


def _prep_in_maps(inputs):
    x = np.asarray(inputs["x"])
    ath, at8 = _pack_a(np.asarray(inputs["w"]))
    in_maps = []
    for c in range(N_CORES):
        xth, xt8 = _pack_x(x[c])
        in_maps.append({"xth": xth, "xt8": xt8, "ath": ath, "at8": at8})
    return in_maps


def kernel(x, w, U, S, V):
    from concourse.bass_utils import run_bass_kernel_spmd

    assert x.shape == (N_CORES, TOK, D)
    nc = _get_compiled()
    in_maps = _prep_in_maps({"x": x, "w": w})

    res = run_bass_kernel_spmd(nc, in_maps, core_ids=list(range(N_CORES)))

    y = np.empty((N_CORES, TOK, D), dtype=np.float32)
    for c in range(N_CORES):
        y[c] = res.results[c]["out"].T
    return y


# revision 20
# speedup vs baseline: 1.1940x; 1.0060x over previous
"""Trainium2 Bass kernel for nn_AstraloraLayer: y = x @ A.T (the low-rank
surrogate path cancels in the forward value).

Sharding: data-parallel over tokens. Each of the 8 cores computes
y[c] = x[c] @ A.T for its [2048, 4096] token shard; A = w.reshape(4096, 4096)
is replicated. No collectives.

Per-core kernel: Y.T[o, t] = sum_k A.T[k, o] * X.T[k, t]. Hybrid precision
over the contraction: k-tiles 0..21 run as fp16 TensorE matmuls (1 cycle/row),
k-tiles 22..31 run as fp8e4 DoubleRow matmuls (2 fp8 weights per PE cell,
0.5 cycles/row). The fp8 operands are pre-scaled (x*8, A*512); the fp16 A is
pre-scaled by 4096 so every matmul accumulates 4096*y in PSUM, and the
PSUM->SBUF drain copy multiplies by 2^-12. Measured rel err ~1.8e-2 vs the
2e-2 gate.

Three token phases (512, 512, 1024) so the first output tiles only wait on a
quarter of X.T; within each output tile the fp8 DoubleRow matmuls (whose
operands land first) run before the fp16 stream. fp16 A.T streams once per
phase, fp8 A.T (4MB) is resident. Warm-up matmuls on scratch SBUF trip the
HAM clock gate before real data lands; the last output tile drains in four
256-token groups across two HWDGE queues to shorten the end-of-kernel tail.
"""

import sys

import numpy as np

if "/opt/trn_rl_repo" not in sys.path:
    sys.path.insert(0, "/opt/trn_rl_repo")

D = 4096          # d_inp == d_out
TOK = 2048        # tokens per core (8 * 2048 total)
N_CORES = 8
P = 128           # partitions
KH = D // P       # 32 k-tiles over the contraction dim
KH16 = 22         # k-tiles 0..21 in fp16
NDR = (KH - KH16) // 2  # 5 DoubleRow pairs for k-tiles 22..31
NOT = D // P      # 32 output tiles
PHASES = [(0, 512), (512, 1536)]
WARMUP_MMS = 44   # scratch matmuls to warm the PE clock before data lands

SX = 8.0          # fp8 x scale
SA = 512.0        # fp8 A scale
SH = SX * SA      # fp16 A pre-scale; PSUM holds SH * y
INV = 1.0 / SH

_COMPILED = None


def _build():
    import concourse.mybir as mybir
    import concourse.tile as tile
    from concourse import bacc

    f16 = mybir.dt.float16
    f8 = mybir.dt.float8e4
    f32 = mybir.dt.float32
    DR = mybir.MatmulPerfMode.DoubleRow

    nc = bacc.Bacc("TRN2", target_bir_lowering=False)

    # xth[p, kh, t] = x[t, kh*128 + p]                          (kh < KH16)
    xth_ext = nc.declare_dram_parameter("xth", [P, KH16, TOK], f16, isOutput=False)
    # xt8[p, j, s, t] = x[t, (KH16+2j+s)*128 + p] * SX
    xt8_ext = nc.declare_dram_parameter("xt8", [P, NDR, 2, TOK], f8, isOutput=False)
    # ath[p, ot, kh, o] = A[ot*128 + o, kh*128 + p] * SH        (kh < KH16)
    ath_ext = nc.declare_dram_parameter("ath", [P, NOT, KH16, P], f16, isOutput=False)
    # at8[p, ot, j, s, o] = A[ot*128 + o, (KH16+2j+s)*128 + p] * SA
    at8_ext = nc.declare_dram_parameter(
        "at8", [P, NOT, NDR, 2, P], f8, isOutput=False
    )
    # out: Y.T [o, t]
    out_ext = nc.declare_dram_parameter("out", [D, TOK], f32, isOutput=True)

    # fp16 X.T chunk plan per phase (kh tiles per chunk)
    CHUNK_PLAN = [[1, 3, 4, 4, 4, 4, 2], [8, 8, 6]]

    with tile.TileContext(nc) as tc:
        with (
            tc.tile_pool(name="wu", bufs=1) as wu_pool,
            tc.tile_pool(name="a8", bufs=1) as a8_pool,
            tc.tile_pool(name="xt", bufs=1) as xt_pool,
            tc.tile_pool(name="at", bufs=4) as at_pool,
            tc.tile_pool(name="ps5", bufs=2, space="PSUM") as ps5_pool,
            tc.tile_pool(name="ps10", bufs=2, space="PSUM") as ps10_pool,
            tc.tile_pool(name="ys", bufs=2) as ys_pool,
        ):
            # Warm-up matmuls on scratch SBUF: keep the PE busy while the
            # first DMAs land, and trip the HAM clock gate to 2.4 GHz
            # before the first real matmul issues.
            wu = wu_pool.tile([P, 256], f16, tag="wu", name="wu")
            nc.vector.memset(wu[:], 0.25)
            wps = ps10_pool.tile([P, 1536], f32, tag="ps10", name="ps10")
            for _ in range(WARMUP_MMS):
                nc.tensor.matmul(
                    wps[:, 0:P], wu[:, 0:P], wu[:, P : 2 * P], start=True, stop=True
                )

            # fp8 A.T is A-only and small (5MB): resident, loaded in 4
            # sub-tiles of 8 ot on the scalar HWDGE queue so the first DR
            # matmuls only wait on 1.25MB. Only sub 0 loads up front; subs
            # 1-3 are deferred into the phase-0 ot loop so they don't steal
            # DMA bandwidth from the fp16 A.T stream during startup.
            a8_subs = []
            for g in range(4):
                s = a8_pool.tile(
                    [P, 8, NDR, 2, P], f8, tag=f"a8g{g}", name=f"a8g{g}"
                )
                if g == 0:
                    nc.scalar.dma_start(
                        out=s[:], in_=at8_ext[:, 0:8, :, :, :]
                    )
                a8_subs.append(s)

            # X.T loads ride the gpsimd DMA queue, separate from the A.T
            # stream on the sync queue. Per phase: the fp8 slice first (the
            # DR matmuls run first within each ot), then the fp16 chunks.
            # Each chunk is its own tile so matmuls only wait on the chunk
            # they actually read.
            xth_sb, xth_map, x8_sb = [], [], []
            for ph, (pt0, ptn) in enumerate(PHASES):
                x8 = xt_pool.tile(
                    [P, NDR, 2, ptn], f8, tag=f"x8p{ph}", name=f"x8p{ph}"
                )
                nc.gpsimd.dma_start(
                    out=x8[:], in_=xt8_ext[:, :, :, pt0 : pt0 + ptn]
                )
                x8_sb.append(x8)
                chunks, kmap, kh0 = [], [], 0
                for c, ch in enumerate(CHUNK_PLAN[ph]):
                    t = xt_pool.tile(
                        [P, ch, ptn], f16, tag=f"xtp{ph}c{c}", name=f"xtp{ph}c{c}"
                    )
                    nc.gpsimd.dma_start(
                        out=t[:], in_=xth_ext[:, kh0 : kh0 + ch, pt0 : pt0 + ptn]
                    )
                    for r in range(ch):
                        kmap.append((c, r))
                    chunks.append(t)
                    kh0 += ch
                xth_sb.append(chunks)
                xth_map.append(kmap)

            for ph, (pt0, ptn) in enumerate(PHASES):
                for ot in range(NOT):
                    # Deferred loads of the remaining resident fp8 A.T subs,
                    # two output-tiles ahead of first use.
                    if ph == 0 and ot in (6, 14, 22):
                        g = ot // 8 + 1
                        nc.scalar.dma_start(
                            out=a8_subs[g][:],
                            in_=at8_ext[:, g * 8 : (g + 1) * 8, :, :, :],
                        )
                    # First fp16 A.T tile of the run arrives kh-sliced so the
                    # first fp16 matmuls only wait on a 64KB load.
                    if ph == 0 and ot == 0:
                        at_subs, kh0 = [], 0
                        for si, ch in enumerate((2, 6, 7, 7)):
                            s = at_pool.tile(
                                [P, ch, P], f16, tag=f"at0s{si}", name="at_s"
                            )
                            nc.sync.dma_start(
                                out=s[:], in_=ath_ext[:, ot, kh0 : kh0 + ch, :]
                            )
                            for r in range(ch):
                                at_subs.append((s, r))
                            kh0 += ch
                    else:
                        at_t = at_pool.tile([P, KH16, P], f16, tag="at", name="at_t")
                        nc.sync.dma_start(out=at_t[:], in_=ath_ext[:, ot, :, :])
                        at_subs = [(at_t, kh) for kh in range(KH16)]
                    a8t = a8_subs[ot // 8]
                    o8 = ot % 8

                    # The very last output tile runs as 4 independent
                    # 256-token accumulation groups so its drain (copy +
                    # store + DMA receipt) pipelines against its own matmuls
                    # instead of all landing after the final matmul.
                    last = ph == len(PHASES) - 1 and ot == NOT - 1
                    if last:
                        GRP = [(0, 512), (512, 512), (1024, 448), (1472, 64)]
                        for g in range(4):
                            t0, t1 = GRP[g][0], GRP[g][0] + GRP[g][1]
                            pst = ps5_pool.tile(
                                [P, t1 - t0], f32, tag="ps5", name="pst"
                            )
                            for j in range(NDR):
                                nc.tensor.matmul(
                                    pst[:],
                                    a8t[:, o8, j, :, :],
                                    x8_sb[ph][:, j, :, t0:t1],
                                    start=(j == 0),
                                    stop=False,
                                    perf_mode=DR,
                                )
                            for kh in range(KH16):
                                c, r = xth_map[ph][kh]
                                a_t, a_r = at_subs[kh]
                                nc.tensor.matmul(
                                    pst[:],
                                    a_t[:, a_r, :],
                                    xth_sb[ph][c][:, r, t0:t1],
                                    start=False,
                                    stop=(kh == KH16 - 1),
                                )
                            ys = ys_pool.tile(
                                [P, t1 - t0], f32, tag="ys", name="ysg"
                            )
                            nc.vector.tensor_scalar_mul(ys[:], pst[:], INV)
                            eng = nc.sync if g % 2 == 1 else nc.scalar
                            eng.dma_start(
                                out=out_ext[
                                    ot * P : (ot + 1) * P, pt0 + t0 : pt0 + t1
                                ],
                                in_=ys[:],
                            )
                        continue

                    pool = ps10_pool if ptn > 512 else ps5_pool
                    ps = pool.tile(
                        [P, ptn], f32, tag=f"ps{10 if ptn > 512 else 5}",
                        name="ps",
                    )
                    # Phase 0 runs the fp8 DoubleRow matmuls first (their
                    # operands land before the fp16 X.T stream); later phases
                    # run fp16 first so each accumulation group opens with
                    # the cheap 128-col LDWEIGHTS (the group-start weight
                    # load doesn't hide behind the previous matmul).
                    def dr_mms(first):
                        for j in range(NDR):
                            for h in range(ptn // 512):
                                nc.tensor.matmul(
                                    ps[:, h * 512 : (h + 1) * 512],
                                    a8t[:, o8, j, :, :],
                                    x8_sb[ph][:, j, :, h * 512 : (h + 1) * 512],
                                    start=(first and j == 0),
                                    stop=(not first and j == NDR - 1),
                                    perf_mode=DR,
                                )

                    def f16_mms(first):
                        for kh in range(KH16):
                            c, r = xth_map[ph][kh]
                            a_t, a_r = at_subs[kh]
                            for h in range(ptn // 512):
                                nc.tensor.matmul(
                                    ps[:, h * 512 : (h + 1) * 512],
                                    a_t[:, a_r, :],
                                    xth_sb[ph][c][:, r, h * 512 : (h + 1) * 512],
                                    start=(first and kh == 0),
                                    stop=(not first and kh == KH16 - 1),
                                )

                    if ph == 0:
                        dr_mms(True)
                        f16_mms(False)
                    else:
                        f16_mms(True)
                        dr_mms(False)
                    # Output drain: the PSUM->SBUF copy applies the 2^-12
                    # descale; stores ride the scalar HWDGE queue (A.T loads
                    # own the sync queue).
                    ys = ys_pool.tile([P, ptn], f32, tag="ys", name=f"ys{ptn}")
                    nc.vector.tensor_scalar_mul(ys[:], ps[:], INV)
                    nc.scalar.dma_start(
                        out=out_ext[ot * P : (ot + 1) * P, pt0 : pt0 + ptn],
                        in_=ys[:],
                    )

    nc.compile()
    return nc


def _get_compiled():
    global _COMPILED
    if _COMPILED is None:
        _COMPILED = _build()
    return _COMPILED


def _f8np():
    import ml_dtypes

    return ml_dtypes.float8_e4m3


def _pack_a(w):
    A = np.asarray(w, dtype=np.float32).reshape(D, D)
    # fp16 part: [p, ot, kh, o] = A[ot*128+o, kh*128+p] * SH for kh < KH16
    Ah = (A[:, : KH16 * P] * SH).reshape(NOT, P, KH16, P)
    ath = np.ascontiguousarray(Ah.transpose(3, 0, 2, 1), dtype=np.float16)
    # fp8 part: [p, ot, j, s, o] = A[ot*128+o, (KH16+2j+s)*128+p] * SA
    A8 = (A[:, KH16 * P :] * SA).reshape(NOT, P, NDR, 2, P)
    at8 = np.ascontiguousarray(A8.transpose(4, 0, 2, 3, 1)).astype(_f8np())
    return ath, at8


def _pack_x(xc):
    xc = np.asarray(xc, dtype=np.float32)
    # fp16 part: [p, kh, t] = x[t, kh*128+p]
    Xh = xc[:, : KH16 * P].reshape(TOK, KH16, P)
    xth = np.ascontiguousarray(Xh.transpose(2, 1, 0), dtype=np.float16)
    # fp8 part: [p, j, s, t]
    X8 = (xc[:, KH16 * P :] * SX).reshape(TOK, NDR, 2, P)
    xt8 = np.ascontiguousarray(X8.transpose(3, 1, 2, 0)).astype(_f8np())
    return xth, xt8


def _prep_in_maps(inputs):
    x = np.asarray(inputs["x"])
    ath, at8 = _pack_a(np.asarray(inputs["w"]))
    in_maps = []
    for c in range(N_CORES):
        xth, xt8 = _pack_x(x[c])
        in_maps.append({"xth": xth, "xt8": xt8, "ath": ath, "at8": at8})
    return in_maps


def kernel(x, w, U, S, V):
    from concourse.bass_utils import run_bass_kernel_spmd

    assert x.shape == (N_CORES, TOK, D)
    nc = _get_compiled()
    in_maps = _prep_in_maps({"x": x, "w": w})

    res = run_bass_kernel_spmd(nc, in_maps, core_ids=list(range(N_CORES)))

    y = np.empty((N_CORES, TOK, D), dtype=np.float32)
    for c in range(N_CORES):
        y[c] = res.results[c]["out"].T
    return y
